# Initial kernel scaffold
#
"""Contact-guided attention augment kernel for 8 trn2 NeuronCores.

Sharding: 8 cores = 4 head-pairs x 2 query-halves.
  core c: g = c % 4 -> heads (2g, 2g+1); sh = c // 4 -> query rows
  [sh*2048, (sh+1)*2048).
Each core's inputs are permuted so its own sequence half comes first,
making the device program identical across cores (pure SPMD).

Device layout: scores are computed transposed (keys on partitions,
queries on free dim), so softmax needs no attention transpose; the
softmax denominator comes from an extra ones-column in the V matmul
(M=65), and normalization folds into the PSUM->SBUF copy of the
attention output.
"""

import numpy as np

H = 8
D = 64
S = 4096
IN = 1024
NODE = 512
EPS = 1e-5

N_CORES = 8
N_G = 4          # head-pair groups
N_SH = 2         # sequence halves
SH_S = S // N_SH # 2048 queries per core
SBW = 1024       # query block width in main loop
N_SB = SH_S // SBW   # 2 query blocks
N_TC = S // 128      # 32 key chunks

_cache = {}


def _build():
    from contextlib import ExitStack

    from concourse import bacc, bass, mybir, tile
    from concourse.masks import make_identity

    f32 = mybir.dt.float32
    f32r = mybir.dt.float32r
    bf16 = mybir.dt.bfloat16
    AF = mybir.ActivationFunctionType

    nc = bacc.Bacc(
        "TRN2",
        target_bir_lowering=False,
        debug=False,
        num_devices=N_CORES,
    )

    embT_d = nc.dram_tensor("embT", [IN, S], bf16, kind="ExternalInput")
    contactT_d = nc.dram_tensor("contactT", [S, SH_S], f32, kind="ExternalInput")
    wqqT_d = nc.dram_tensor("wqqT", [IN, 128], bf16, kind="ExternalInput")
    wkkT_d = nc.dram_tensor("wkkT", [IN, 128], bf16, kind="ExternalInput")
    wvvT_d = nc.dram_tensor("wvvT", [IN, 128], bf16, kind="ExternalInput")
    qscale_d = nc.dram_tensor("qscale", [128, 1], f32, kind="ExternalInput")
    qbias_d = nc.dram_tensor("qbias", [128, 1], f32, kind="ExternalInput")
    kbias_d = nc.dram_tensor("kbias", [128, 1], f32, kind="ExternalInput")
    vbias_d = nc.dram_tensor("vbias", [128, 1], f32, kind="ExternalInput")
    woTp_d = nc.dram_tensor("woTp", [64, 8, NODE], f32r, kind="ExternalInput")
    vecs_d = nc.dram_tensor("vecs", [3, NODE], f32, kind="ExternalInput")
    emb_res_d = nc.dram_tensor("emb_res", [NODE, NODE], f32, kind="ExternalInput")
    ones_d = nc.dram_tensor("ones", [128, 64], bf16, kind="ExternalInput")
    out_d = nc.dram_tensor("out", [NODE, NODE], f32, kind="ExternalOutput")

    def bcast(ap, n):
        return bass.AP(tensor=ap.tensor, offset=ap.offset, ap=[[0, n]] + list(ap.ap))

    with tile.TileContext(nc) as tc, ExitStack() as ctx:
        consts = ctx.enter_context(tc.tile_pool(name="consts", bufs=1))
        acts = ctx.enter_context(tc.tile_pool(name="acts", bufs=1))

        wqq = consts.tile([128, 8, 128], bf16)
        nc.sync.dma_start(out=wqq, in_=wqqT_d.ap().rearrange("(c p) m -> p c m", p=128))
        wkk = consts.tile([128, 8, 128], bf16)
        nc.sync.dma_start(out=wkk, in_=wkkT_d.ap().rearrange("(c p) m -> p c m", p=128))
        wvv = consts.tile([128, 8, 128], bf16)
        nc.sync.dma_start(out=wvv, in_=wvvT_d.ap().rearrange("(c p) m -> p c m", p=128))
        woTp = consts.tile([64, 8, NODE], f32r)
        nc.sync.dma_start(out=woTp, in_=woTp_d.ap())
        qscale = consts.tile([128, 1], f32)
        nc.sync.dma_start(out=qscale, in_=qscale_d.ap())
        qbias = consts.tile([128, 1], f32)
        nc.sync.dma_start(out=qbias, in_=qbias_d.ap())
        kbias = consts.tile([128, 1], f32)
        nc.sync.dma_start(out=kbias, in_=kbias_d.ap())
        vbias = consts.tile([128, 1], f32)
        nc.sync.dma_start(out=vbias, in_=vbias_d.ap())
        vecs = consts.tile([128, 3, NODE], f32)
        nc.sync.dma_start(out=vecs, in_=bcast(vecs_d.ap(), 128))
        emb_res = consts.tile([128, 4, NODE], f32)
        nc.sync.dma_start(
            out=emb_res, in_=emb_res_d.ap().rearrange("(c p) n -> p c n", p=128)
        )
        ident = consts.tile([128, 128], f32)
        make_identity(nc, ident)

        qT = acts.tile([128, SH_S], bf16)       # both heads stacked on partitions
        kT = acts.tile([128, S], bf16)
        v_nat = acts.tile([128, N_TC, 130], bf16)  # [t, chunk, (V_h0|1|V_h1|1)]
        x_all = acts.tile([128, 4, NODE], f32)    # pre-layernorm rows

        embT_r = embT_d.ap().rearrange("(c p) s -> p c s", p=128)

        # ---- phase A: projections ----
        with tc.tile_pool(name="pa_in", bufs=2) as pa_in, \
             tc.tile_pool(name="pa_ps", bufs=4, space="PSUM") as pa_ps, \
             tc.tile_pool(name="vT_pool", bufs=1) as vT_pool, \
             tc.tile_pool(name="pt_ps", bufs=2, space="PSUM") as pt_ps:
            vT = vT_pool.tile([128, S], f32)
            for sc in range(8):
                et = pa_in.tile([128, 8, 512], bf16)
                nc.sync.dma_start(out=et, in_=embT_r[:, :, sc * 512:(sc + 1) * 512])
                pk = pa_ps.tile([128, 512], f32, tag="pa", name="pk")
                pv = pa_ps.tile([128, 512], f32, tag="pa", name="pv")
                pq = None
                if sc < 4:
                    pq = pa_ps.tile([128, 512], f32, tag="pa", name="pq")
                for kc in range(8):
                    st, sp = kc == 0, kc == 7
                    nc.tensor.matmul(pk, wkk[:, kc, :],
                                     et[:, kc, :], start=st, stop=sp)
                    nc.tensor.matmul(pv, wvv[:, kc, :],
                                     et[:, kc, :], start=st, stop=sp)
                    if pq is not None:
                        nc.tensor.matmul(pq, wqq[:, kc, :],
                                         et[:, kc, :], start=st, stop=sp)
                sl = slice(sc * 512, (sc + 1) * 512)
                nc.scalar.activation(out=kT[:, sl], in_=pk, func=AF.Identity,
                                     bias=kbias)
                nc.scalar.activation(out=vT[:, sl], in_=pv, func=AF.Identity,
                                     bias=vbias)
                if pq is not None:
                    nc.scalar.activation(out=qT[:, sl], in_=pq, func=AF.Identity,
                                         bias=qbias, scale=qscale)

            # V into natural layout [t, d] (both heads), plus ones columns
            ones_r = ones_d.ap().rearrange("p (c one) -> p c one", one=1)
            nc.sync.dma_start(out=v_nat[:, :, 64:65], in_=ones_r[:, 0:N_TC, :])
            nc.sync.dma_start(out=v_nat[:, :, 129:130], in_=ones_r[:, 32:32 + N_TC, :])
            for blk in range(N_TC):
                pvt = pt_ps.tile([128, 128], f32)
                nc.tensor.transpose(pvt, vT[:, blk * 128:(blk + 1) * 128], ident)
                nc.scalar.activation(out=v_nat[:, blk, 0:64], in_=pvt[:, 0:64],
                                     func=AF.Copy)
                nc.scalar.activation(out=v_nat[:, blk, 65:129], in_=pvt[:, 64:128],
                                     func=AF.Copy)

        contactT_r = contactT_d.ap().rearrange("(c p) s -> p c s", p=128)

        # ---- phase B: attention ----
        with tc.tile_pool(name="ct", bufs=3) as ct_pool, \
             tc.tile_pool(name="exp0", bufs=2) as exp0, \
             tc.tile_pool(name="exp1", bufs=2) as exp1, \
             tc.tile_pool(name="ein0", bufs=2) as ein0, \
             tc.tile_pool(name="ein1", bufs=2) as ein1, \
             tc.tile_pool(name="ps", bufs=2, space="PSUM") as ps_pool, \
             tc.tile_pool(name="po", bufs=2, space="PSUM") as po_pool, \
             tc.tile_pool(name="dscr", bufs=2, space="DRAM") as dscr, \
             tc.tile_pool(name="fin", bufs=2) as fin:
            out_r = out_d.ap().rearrange("(c p) n -> p c n", p=128)

            def make_finalize(sb, oc):
                def fz():
                    for hi in range(2):
                        d1 = dscr.tile([1, SBW], f32, tag="d1", name="d1")
                        nc.sync.dma_start(out=d1, in_=oc[hi][64:65, :])
                        r8 = fin.tile([128, 8], f32, tag="r8", name="r8")
                        nc.sync.dma_start(
                            out=r8, in_=d1.rearrange("o (p f) -> (o p) f", p=128))
                        rc8 = fin.tile([128, 8], f32, tag="rc8", name="rc8")
                        nc.vector.reciprocal(out=rc8, in_=r8)
                        d2 = dscr.tile([1, SBW], f32, tag="d2", name="d2")
                        nc.sync.dma_start(
                            out=d2.rearrange("o (p f) -> (o p) f", p=128), in_=rc8)
                        rb = fin.tile([64, SBW], f32, tag="rb", name="rb")
                        nc.sync.dma_start(
                            out=rb,
                            in_=bass.AP(tensor=d2.tensor, offset=d2.offset,
                                        ap=[[0, 64]] + list(d2.ap)[1:]))
                        onorm = fin.tile([64, SBW], f32r, tag="onorm", name="onorm")
                        nc.vector.tensor_mul(onorm, oc[hi][0:64, :], rb)
                        aug = ps_pool.tile([128, 512], f32, tag="ps", name="aug")
                        on_r = onorm.rearrange("p (r j) -> p j r", j=8)
                        for j1 in range(8):
                            nc.tensor.matmul(
                                aug,
                                on_r[:, j1, :],
                                woTp[:, j1, :],
                                start=(j1 == 0), stop=(j1 == 7),
                                skip_group_check=True,
                            )
                        c2 = hi * 2 + sb
                        nc.vector.tensor_add(x_all[:, c2, :], aug, emb_res[:, c2, :])
                        nc.vector.tensor_add(x_all[:, c2, :], x_all[:, c2, :],
                                             vecs[:, 0, :])
                        # layernorm + store for this block
                        stats = fin.tile([128, 6], f32, tag="stats", name="stats")
                        nc.vector.bn_stats(stats, x_all[:, c2, :])
                        mv = fin.tile([128, 2], f32, tag="mv", name="mv")
                        nc.vector.bn_aggr(mv, stats)
                        vv = fin.tile([128, 1], f32, tag="vv", name="vv")
                        nc.vector.tensor_scalar_add(vv, mv[:, 1:2], EPS)
                        sq = fin.tile([128, 1], f32, tag="sq", name="sq")
                        nc.scalar.activation(out=sq, in_=vv, func=AF.Sqrt)
                        rstd = fin.tile([128, 1], f32, tag="rstd", name="rstd")
                        nc.vector.reciprocal(out=rstd, in_=sq)
                        t0 = fin.tile([128, 1], f32, tag="t0", name="t0")
                        nc.vector.tensor_mul(t0, rstd, rstd)
                        nc.vector.tensor_mul(t0, t0, vv)
                        nc.vector.tensor_scalar_mul(t0, t0, -0.5)
                        nc.vector.tensor_scalar_add(t0, t0, 1.5)
                        nc.vector.tensor_mul(rstd, rstd, t0)
                        nmu = fin.tile([128, 1], f32, tag="nmu", name="nmu")
                        nc.vector.tensor_mul(nmu, mv[:, 0:1], rstd)
                        nc.vector.tensor_scalar_mul(nmu, nmu, -1.0)
                        xn = fin.tile([128, NODE], f32, tag="xn", name="xn")
                        nc.scalar.activation(out=xn, in_=x_all[:, c2, :],
                                             func=AF.Identity, bias=nmu, scale=rstd)
                        nc.vector.tensor_mul(xn, xn, vecs[:, 1, :])
                        nc.vector.tensor_add(xn, xn, vecs[:, 2, :])
                        nc.sync.dma_start(out=out_r[:, c2, :], in_=xn)
                return fz

            pending = []
            for sb in range(N_SB):
                po = [po_pool.tile([65, SBW], f32, tag="po", name=f"po{h}")
                      for h in range(2)]
                ex = [None, None]
                ei = [None, None]
                for tcx in range(N_TC):
                    ct = ct_pool.tile([128, SBW], f32)
                    nc.sync.dma_start(
                        out=ct,
                        in_=contactT_r[:, tcx, sb * SBW:(sb + 1) * SBW],
                    )
                    parity = tcx % 2
                    for hi in range(2):
                        hp = slice(hi * 64, (hi + 1) * 64)
                        if parity == 0:
                            ei[hi] = (ein0 if hi == 0 else ein1).tile(
                                [128, 2 * SBW], f32, name=f"ei{hi}")
                            ex[hi] = (exp0 if hi == 0 else exp1).tile(
                                [128, 2 * SBW], bf16, name=f"ex{hi}")
                        pss = ps_pool.tile([128, SBW], f32, tag="ps")
                        for half in range(2):
                            nc.tensor.matmul(
                                pss[:, half * 512:(half + 1) * 512],
                                kT[hp, tcx * 128:(tcx + 1) * 128],
                                qT[hp, sb * SBW + half * 512: sb * SBW + (half + 1) * 512],
                                start=True, stop=True, skip_group_check=True,
                            )
                        nc.vector.tensor_mul(
                            ei[hi][:, parity * SBW:(parity + 1) * SBW], pss, ct
                        )
                        if parity == 1:
                            nc.scalar.activation(out=ex[hi], in_=ei[hi], func=AF.Exp)
                            for t2 in (tcx - 1, tcx):
                                for half in range(2):
                                    nc.tensor.matmul(
                                        po[hi][:, half * 512:(half + 1) * 512],
                                        v_nat[:, t2, hi * 65:(hi + 1) * 65],
                                        ex[hi][:, (t2 % 2) * SBW + half * 512:
                                               (t2 % 2) * SBW + (half + 1) * 512],
                                        start=(t2 == 0), stop=(t2 == N_TC - 1),
                                        skip_group_check=True,
                                    )
                    if tcx == 8 and pending:
                        for fz in pending:
                            fz()
                        pending = []
                # release PSUM fast: copy po to SBUF, defer everything else
                oc = []
                for hi in range(2):
                    t = fin.tile([65, SBW], f32, tag=f"oc{hi}", name=f"oc{hi}")
                    nc.scalar.activation(out=t, in_=po[hi][0:65, :], func=AF.Copy)
                    oc.append(t)
                pending.append(make_finalize(sb, oc))
            for fz in pending:
                fz()

    nc.compile()
    return nc


def make_in_maps(protT5_emb, contact_matrix, Wq, bq, Wk, bk, Wv, bv, Wc, Wo, bo,
                 gamma, beta):
    """Host-side sharding: slices, transposes, concats only (plus O(H)
    scalar constant folding for the Wc/sqrt(D) score scale)."""
    emb = np.asarray(protT5_emb, np.float32)
    contact = np.asarray(contact_matrix, np.float32)
    wc = np.asarray(Wc, np.float32).reshape(H)
    inv = 1.0 / np.sqrt(np.float32(D))

    embT = np.ascontiguousarray(emb.T)  # [IN, S]
    woTp = np.ascontiguousarray(
        np.asarray(Wo, np.float32).T.reshape(8, 64, NODE).transpose(1, 0, 2)
    )
    vecs = np.ascontiguousarray(np.stack([bo, gamma, beta]).astype(np.float32))

    # per-sh permuted embT and contactT
    embT_sh = []
    contactT_sh = []
    for sh in range(N_SH):
        own = slice(sh * SH_S, (sh + 1) * SH_S)
        oth = slice((1 - sh) * SH_S, (2 - sh) * SH_S)
        embT_sh.append(np.ascontiguousarray(
            np.concatenate([embT[:, own], embT[:, oth]], axis=1)))
        A = contact[own, :]  # [SH_S rows s, S cols t]
        contactT_sh.append(np.ascontiguousarray(
            np.concatenate([A[:, own].T, A[:, oth].T], axis=0)))

    in_maps = []
    for c in range(N_CORES):
        g, sh = c % N_G, c // N_G
        h0, h1 = 2 * g, 2 * g + 1
        s0, s1 = slice(h0 * D, (h0 + 1) * D), slice(h1 * D, (h1 + 1) * D)
        wqqT = np.ascontiguousarray(np.concatenate([Wq[s0], Wq[s1]]).T)
        wkkT = np.ascontiguousarray(np.concatenate([Wk[s0], Wk[s1]]).T)
        wvvT = np.ascontiguousarray(np.concatenate([Wv[s0], Wv[s1]]).T)
        qscale = np.concatenate([
            np.full(D, wc[h0] * inv, np.float32),
            np.full(D, wc[h1] * inv, np.float32)])[:, None]
        qbias = np.concatenate([
            np.asarray(bq, np.float32)[s0] * (wc[h0] * inv),
            np.asarray(bq, np.float32)[s1] * (wc[h1] * inv)])[:, None]
        kbias = np.concatenate([np.asarray(bk, np.float32)[s0],
                                np.asarray(bk, np.float32)[s1]])[:, None]
        vbias = np.concatenate([np.asarray(bv, np.float32)[s0],
                                np.asarray(bv, np.float32)[s1]])[:, None]
        r0 = slice(h0 * NODE + sh * 256, h0 * NODE + (sh + 1) * 256)
        r1 = slice(h1 * NODE + sh * 256, h1 * NODE + (sh + 1) * 256)
        emb_res = np.ascontiguousarray(
            np.concatenate([emb[r0, :NODE], emb[r1, :NODE]]))
        bf = __import__("ml_dtypes").bfloat16
        in_maps.append({
            "embT": embT_sh[sh].astype(bf),
            "contactT": contactT_sh[sh],
            "wqqT": wqqT.astype(bf),
            "wkkT": wkkT.astype(bf),
            "wvvT": wvvT.astype(bf),
            "qscale": np.ascontiguousarray(qscale),
            "qbias": np.ascontiguousarray(qbias),
            "kbias": np.ascontiguousarray(kbias),
            "vbias": np.ascontiguousarray(vbias),
            "woTp": woTp,
            "vecs": vecs,
            "emb_res": emb_res,
            "ones": np.ones((128, 64), __import__("ml_dtypes").bfloat16),
        })
    return in_maps


def assemble(results):
    out = np.empty((S, NODE), np.float32)
    for c in range(N_CORES):
        g, sh = c % N_G, c // N_G
        h0, h1 = 2 * g, 2 * g + 1
        blk = results[c]["out"]
        out[h0 * NODE + sh * 256: h0 * NODE + (sh + 1) * 256] = blk[:256]
        out[h1 * NODE + sh * 256: h1 * NODE + (sh + 1) * 256] = blk[256:]
    return out


def kernel(**inputs):
    from concourse.bass_utils import run_bass_kernel_spmd

    if "nc" not in _cache:
        _cache["nc"] = _build()
    nc = _cache["nc"]
    in_maps = make_in_maps(**inputs)
    res = run_bass_kernel_spmd(nc, in_maps, list(range(N_CORES)))
    return assemble(res.results)



# revision 17
# speedup vs baseline: 1.0696x; 1.0696x over previous
"""Contact-guided attention augment kernel for 8 trn2 NeuronCores.

Sharding: 8 cores = 4 head-pairs x 2 query-halves.
  core c: g = c % 4 -> heads (2g, 2g+1); sh = c // 4 -> query rows
  [sh*2048, (sh+1)*2048).
Each core's inputs are permuted so its own sequence half comes first,
making the device program identical across cores (pure SPMD).

Device layout: scores are computed transposed (keys on partitions,
queries on free dim), so softmax needs no attention transpose; the
softmax denominator comes from an extra ones-column in the V matmul
(M=65), and normalization folds into the attention-output path.

v2 notes vs v1:
- contact streamed as bf16 (halves DMA), scores*contact and exp run in
  bf16 tiles; exp batched over 2 key chunks (FD=2048 -- larger batches
  idle the PE >3.4us per cycle and HAM-throttle it to 1.2GHz).
- score matmuls for the two heads interleave so their K=64 row groups
  (tile_position (0,0)/(64,0)) execute concurrently on the PE.
- a fraction of score chunks (1/B_EVERY) detour via a ScalarE
  PSUM->SBUF bf16 copy so the DVE multiply runs in 2x mode --
  balances DVE vs ScalarE load.
- Wo matmul in bf16; emb residual + bo folded host-side; finalize
  elementwise (onorm, gamma/beta) moved to GpSimd.
- embedding chunks DMA'd from a host-prearranged contiguous layout;
  finalize-only constants loaded after phase A.
"""

import numpy as np

H = 8
D = 64
S = 4096
IN = 1024
NODE = 512
EPS = 1e-5

N_CORES = 8
N_G = 4          # head-pair groups
N_SH = 2         # sequence halves
SH_S = S // N_SH # 2048 queries per core
SBW = 1024       # query block width in main loop
N_SB = SH_S // SBW   # 2 query blocks
N_TC = S // 128      # 32 key chunks
B_EVERY = 8          # every B_EVERY-th (tcx,hi) unit takes the ACT-copy path

_cache = {}


def _build():
    from contextlib import ExitStack

    from concourse import bacc, bass, mybir, tile
    from concourse.masks import make_identity

    f32 = mybir.dt.float32
    bf16 = mybir.dt.bfloat16
    AF = mybir.ActivationFunctionType

    nc = bacc.Bacc(
        "TRN2",
        target_bir_lowering=False,
        debug=False,
        num_devices=N_CORES,
    )

    embc_d = nc.dram_tensor("embc", [128, 8, 8 * 512], bf16, kind="ExternalInput")
    contactT_d = nc.dram_tensor("contactT", [S, SH_S], bf16, kind="ExternalInput")
    wqqT_d = nc.dram_tensor("wqqT", [IN, 128], bf16, kind="ExternalInput")
    wkkT_d = nc.dram_tensor("wkkT", [IN, 128], bf16, kind="ExternalInput")
    wvvT_d = nc.dram_tensor("wvvT", [IN, 128], bf16, kind="ExternalInput")
    qscale_d = nc.dram_tensor("qscale", [128, 1], f32, kind="ExternalInput")
    qbias_d = nc.dram_tensor("qbias", [128, 1], f32, kind="ExternalInput")
    kbias_d = nc.dram_tensor("kbias", [128, 1], f32, kind="ExternalInput")
    vbias_d = nc.dram_tensor("vbias", [128, 1], f32, kind="ExternalInput")
    woTp_d = nc.dram_tensor("woTp", [64, 8, NODE], bf16, kind="ExternalInput")
    vecs_d = nc.dram_tensor("vecs", [2, NODE], f32, kind="ExternalInput")
    emb_res_d = nc.dram_tensor("emb_res", [NODE, NODE], f32, kind="ExternalInput")
    out_d = nc.dram_tensor("out", [NODE, NODE], f32, kind="ExternalOutput")

    def bcast(ap, n):
        return bass.AP(tensor=ap.tensor, offset=ap.offset, ap=[[0, n]] + list(ap.ap))

    with tile.TileContext(nc) as tc, ExitStack() as ctx:
        consts = ctx.enter_context(tc.tile_pool(name="consts", bufs=1))
        acts = ctx.enter_context(tc.tile_pool(name="acts", bufs=1))

        # weights + biases first (small), then phase A streams embeddings;
        # finalize-only constants are DMA'd after phase A is issued.
        wqq = consts.tile([128, 8, 128], bf16)
        nc.sync.dma_start(out=wqq, in_=wqqT_d.ap().rearrange("(c p) m -> p c m", p=128))
        wkk = consts.tile([128, 8, 128], bf16)
        nc.sync.dma_start(out=wkk, in_=wkkT_d.ap().rearrange("(c p) m -> p c m", p=128))
        wvv = consts.tile([128, 8, 128], bf16)
        nc.sync.dma_start(out=wvv, in_=wvvT_d.ap().rearrange("(c p) m -> p c m", p=128))
        ident = consts.tile([128, 128], f32)
        make_identity(nc, ident)
        qscale = consts.tile([128, 1], f32)
        qbias = consts.tile([128, 1], f32)
        kbias = consts.tile([128, 1], f32)
        vbias = consts.tile([128, 1], f32)

        qT = acts.tile([128, SH_S], bf16)       # both heads stacked on partitions
        kT = acts.tile([128, S], bf16)
        v_nat = acts.tile([128, N_TC, 130], bf16)  # [t, chunk, (V_h0|1|V_h1|1)]
        x_all = acts.tile([128, 4, NODE], f32)    # pre-layernorm rows

        # ---- phase A: projections ----
        with tc.tile_pool(name="pa_in", bufs=2) as pa_in, \
             tc.tile_pool(name="pa_ps", bufs=4, space="PSUM") as pa_ps, \
             tc.tile_pool(name="vT_pool", bufs=1) as vT_pool, \
             tc.tile_pool(name="pt_ps", bufs=2, space="PSUM") as pt_ps:
            vT = vT_pool.tile([128, S], bf16)
            identb = vT_pool.tile([128, 128], bf16)
            for sc in range(8):
                et = pa_in.tile([128, 8, 512], bf16)
                nc.sync.dma_start(out=et, in_=embc_d.ap()[:, sc, :])
                if sc == 0:
                    # small consts after the first embedding chunk is on
                    # the queue (they gate only the PSUM->SBUF copies)
                    nc.sync.dma_start(out=qscale, in_=qscale_d.ap())
                    nc.sync.dma_start(out=qbias, in_=qbias_d.ap())
                    nc.sync.dma_start(out=kbias, in_=kbias_d.ap())
                    nc.sync.dma_start(out=vbias, in_=vbias_d.ap())
                    nc.vector.tensor_copy(identb, ident)
                    nc.gpsimd.memset(v_nat[:, :, 64:65], 1.0)
                    nc.gpsimd.memset(v_nat[:, :, 129:130], 1.0)
                pk = pa_ps.tile([128, 512], f32, tag="pa", name="pk")
                pv = pa_ps.tile([128, 512], f32, tag="pa", name="pv")
                pq = None
                if sc < 4:
                    pq = pa_ps.tile([128, 512], f32, tag="pa", name="pq")
                for kc in range(8):
                    st, sp = kc == 0, kc == 7
                    nc.tensor.matmul(pk, wkk[:, kc, :],
                                     et[:, kc, :], start=st, stop=sp)
                    nc.tensor.matmul(pv, wvv[:, kc, :],
                                     et[:, kc, :], start=st, stop=sp)
                    if pq is not None:
                        nc.tensor.matmul(pq, wqq[:, kc, :],
                                         et[:, kc, :], start=st, stop=sp)
                sl = slice(sc * 512, (sc + 1) * 512)
                nc.scalar.activation(out=kT[:, sl], in_=pk, func=AF.Identity,
                                     bias=kbias)
                nc.scalar.activation(out=vT[:, sl], in_=pv, func=AF.Identity,
                                     bias=vbias)
                if pq is not None:
                    nc.scalar.activation(out=qT[:, sl], in_=pq, func=AF.Identity,
                                         bias=qbias, scale=qscale)
                # V chunks of this sc into natural layout right away --
                # keeps the PE stream dense (transposes at the tail of
                # phase A otherwise spill into attention at cold clock)
                for blk in range(sc * 4, sc * 4 + 4):
                    pvt = pt_ps.tile([128, 128], bf16)
                    nc.tensor.transpose(pvt, vT[:, blk * 128:(blk + 1) * 128],
                                        identb)
                    nc.scalar.activation(out=v_nat[:, blk, 0:64], in_=pvt[:, 0:64],
                                         func=AF.Copy)
                    nc.scalar.activation(out=v_nat[:, blk, 65:129],
                                         in_=pvt[:, 64:128], func=AF.Copy)

        # finalize-only constants (issued after phase A on the sync queue)
        woTp = consts.tile([64, 8, NODE], bf16)
        nc.sync.dma_start(out=woTp, in_=woTp_d.ap())
        vecs = consts.tile([128, 2, NODE], f32)
        nc.sync.dma_start(out=vecs, in_=bcast(vecs_d.ap(), 128))
        emb_res = consts.tile([128, 4, NODE], f32)
        nc.sync.dma_start(
            out=emb_res, in_=emb_res_d.ap().rearrange("(c p) n -> p c n", p=128)
        )

        contactT_r = contactT_d.ap().rearrange("(c p) s -> p c s", p=128)

        # ---- phase B: attention ----
        with tc.tile_pool(name="ct", bufs=3) as ct_pool, \
             tc.tile_pool(name="exp0", bufs=2) as exp0, \
             tc.tile_pool(name="exp1", bufs=2) as exp1, \
             tc.tile_pool(name="ein0", bufs=2) as ein0, \
             tc.tile_pool(name="ein1", bufs=2) as ein1, \
             tc.tile_pool(name="sb0", bufs=2) as sbp0, \
             tc.tile_pool(name="sb1", bufs=2) as sbp1, \
             tc.tile_pool(name="ps", bufs=2, space="PSUM") as ps_pool, \
             tc.tile_pool(name="po", bufs=2, space="PSUM") as po_pool, \
             tc.tile_pool(name="dscr", bufs=2, space="DRAM") as dscr, \
             tc.tile_pool(name="fin", bufs=2) as fin:
            out_r = out_d.ap().rearrange("(c p) n -> p c n", p=128)

            def make_finalize(sb, oc):
                """Return a list of small thunks (spread one-per-tcx so the
                finalize never starves the PE long enough to HAM-throttle).
                LN rsqrt is DVE-only (Pade seed + 2 Newton steps) -- an ACT
                Sqrt would thrash the exp activation-table set."""
                st = {}

                def p_den(hi):
                    def f():
                        d1 = dscr.tile([1, SBW], f32, tag="d1", name="d1")
                        nc.sync.dma_start(out=d1, in_=oc[hi][64:65, :])
                        r8 = fin.tile([128, 8], f32, tag="r8", name="r8")
                        nc.sync.dma_start(
                            out=r8, in_=d1.rearrange("o (p f) -> (o p) f", p=128))
                        rc8 = fin.tile([128, 8], f32, tag="rc8", name="rc8")
                        nc.vector.reciprocal(out=rc8, in_=r8)
                        d2 = dscr.tile([1, SBW], f32, tag="d2", name="d2")
                        nc.sync.dma_start(
                            out=d2.rearrange("o (p f) -> (o p) f", p=128), in_=rc8)
                        rb = fin.tile([64, SBW], f32, tag="rb", name="rb")
                        nc.sync.dma_start(
                            out=rb,
                            in_=bass.AP(tensor=d2.tensor, offset=d2.offset,
                                        ap=[[0, 64]] + list(d2.ap)[1:]))
                        st[f"rb{hi}"] = rb
                    return f

                def p_onorm(hi):
                    def f():
                        onorm = fin.tile([64, SBW], bf16, tag="onorm",
                                         name="onorm")
                        nc.gpsimd.tensor_mul(onorm, oc[hi][0:64, :], st[f"rb{hi}"])
                        st[f"on{hi}"] = onorm
                    return f

                def p_wo(hi):
                    def f():
                        aug = ps_pool.tile([128, 512], f32, tag="ps", name="aug")
                        on_r = st[f"on{hi}"].rearrange("p (r j) -> p j r", j=8)
                        for j1 in range(8):
                            nc.tensor.matmul(
                                aug, on_r[:, j1, :], woTp[:, j1, :],
                                start=(j1 == 0), stop=(j1 == 7),
                                skip_group_check=True,
                            )
                        c2 = hi * 2 + sb
                        nc.vector.tensor_add(x_all[:, c2, :], aug,
                                             emb_res[:, c2, :])
                    return f

                def p_stats(hi):
                    def f():
                        c2 = hi * 2 + sb
                        stats = fin.tile([128, 6], f32, tag="stats", name="stats")
                        nc.vector.bn_stats(stats, x_all[:, c2, :])
                        mv = fin.tile([128, 2], f32, tag="mv", name="mv")
                        nc.vector.bn_aggr(mv, stats)
                        st[f"mv{hi}"] = mv
                    return f

                def p_rstd(hi):
                    def f():
                        mv = st[f"mv{hi}"]
                        vv = fin.tile([128, 1], f32, tag="vv", name="vv")
                        nc.vector.tensor_scalar_add(vv, mv[:, 1:2], EPS)
                        a = fin.tile([128, 1], f32, tag="a", name="a")
                        nc.vector.tensor_scalar_add(a, vv, 1.0)
                        r = fin.tile([128, 1], f32, tag="r", name="r")
                        nc.vector.reciprocal(out=r, in_=a)
                        nc.vector.tensor_scalar_mul(r, r, 2.0)
                        t0 = fin.tile([128, 1], f32, tag="t0", name="t0")
                        for _ in range(2):
                            nc.vector.tensor_mul(t0, r, r)
                            nc.vector.tensor_mul(t0, t0, vv)
                            nc.vector.tensor_scalar(t0, t0, -0.5, 1.5,
                                                    mybir.AluOpType.mult,
                                                    mybir.AluOpType.add)
                            nc.vector.tensor_mul(r, r, t0)
                        nmu = fin.tile([128, 1], f32, tag="nmu", name="nmu")
                        nc.vector.tensor_mul(nmu, mv[:, 0:1], r)
                        nc.vector.tensor_scalar_mul(nmu, nmu, -1.0)
                        st[f"rstd{hi}"] = r
                        st[f"nmu{hi}"] = nmu
                    return f

                def p_out(hi):
                    def f():
                        c2 = hi * 2 + sb
                        xn = fin.tile([128, NODE], f32, tag="xn", name="xn")
                        nc.scalar.activation(out=xn, in_=x_all[:, c2, :],
                                             func=AF.Identity,
                                             bias=st[f"nmu{hi}"],
                                             scale=st[f"rstd{hi}"])
                        nc.gpsimd.tensor_mul(xn, xn, vecs[:, 0, :])
                        nc.gpsimd.tensor_add(xn, xn, vecs[:, 1, :])
                        nc.sync.dma_start(out=out_r[:, c2, :], in_=xn)
                    return f

                return [p_den(0), p_den(1), p_onorm(0), p_onorm(1),
                        p_wo(0), p_stats(0), p_wo(1), p_stats(1),
                        p_rstd(0), p_rstd(1), p_out(0), p_out(1)]

            pending = []
            for sb in range(N_SB):
                po = [po_pool.tile([65, SBW], f32, tag="po", name=f"po{h}")
                      for h in range(2)]
                ex = [None, None]
                ei = [None, None]
                for tcx in range(N_TC):
                    ct = ct_pool.tile([128, SBW], bf16)
                    nc.sync.dma_start(
                        out=ct,
                        in_=contactT_r[:, tcx, sb * SBW:(sb + 1) * SBW],
                    )
                    slot = tcx % 2
                    if slot == 0:
                        for hi in range(2):
                            ei[hi] = (ein0 if hi == 0 else ein1).tile(
                                [128, 2 * SBW], bf16, name=f"ei{hi}")
                            ex[hi] = (exp0 if hi == 0 else exp1).tile(
                                [128, 2 * SBW], bf16, name=f"ex{hi}")
                    pss = [ps_pool.tile([128, SBW], f32, tag="ps", name=f"pss{h}")
                           for h in range(2)]
                    # interleave heads so row groups (0,0)/(64,0) overlap on PE
                    for half in range(2):
                        for hi in range(2):
                            hp = slice(hi * 64, (hi + 1) * 64)
                            nc.tensor.matmul(
                                pss[hi][:, half * 512:(half + 1) * 512],
                                kT[hp, tcx * 128:(tcx + 1) * 128],
                                qT[hp, sb * SBW + half * 512: sb * SBW + (half + 1) * 512],
                                start=True, stop=True, skip_group_check=True,
                            )
                    for hi in range(2):
                        dst = ei[hi][:, slot * SBW:(slot + 1) * SBW]
                        if (tcx * 2 + hi) % B_EVERY == B_EVERY - 1:
                            # ACT copy path: frees DVE via 2x bf16 multiply
                            sB = (sbp0 if hi == 0 else sbp1).tile(
                                [128, SBW], bf16, name=f"sB{hi}")
                            nc.scalar.activation(out=sB, in_=pss[hi], func=AF.Copy)
                            nc.vector.tensor_mul(dst, sB, ct)
                        else:
                            nc.vector.tensor_mul(dst, pss[hi], ct)
                        if slot == 1:
                            nc.scalar.activation(out=ex[hi], in_=ei[hi], func=AF.Exp)
                            for t2 in range(tcx - 1, tcx + 1):
                                for half in range(2):
                                    nc.tensor.matmul(
                                        po[hi][:, half * 512:(half + 1) * 512],
                                        v_nat[:, t2, hi * 65:(hi + 1) * 65],
                                        ex[hi][:, (t2 % 2) * SBW + half * 512:
                                               (t2 % 2) * SBW + (half + 1) * 512],
                                        start=(t2 == 0), stop=(t2 == N_TC - 1),
                                        skip_group_check=True,
                                    )
                    if pending and tcx >= 4:
                        pending.pop(0)()
                # release PSUM fast: copy po to SBUF, defer everything else
                oc = []
                for hi in range(2):
                    t = fin.tile([65, SBW], f32, tag=f"oc{hi}", name=f"oc{hi}")
                    nc.scalar.activation(out=t, in_=po[hi][0:65, :], func=AF.Copy)
                    oc.append(t)
                pending = make_finalize(sb, oc)
            for fz in pending:
                fz()

    nc.compile()
    return nc


def make_in_maps(protT5_emb, contact_matrix, Wq, bq, Wk, bk, Wv, bv, Wc, Wo, bo,
                 gamma, beta):
    """Host-side sharding: slices, transposes, concats only (plus O(H)
    scalar constant folding for the Wc/sqrt(D) score scale)."""
    import ml_dtypes
    bf = ml_dtypes.bfloat16

    emb = np.asarray(protT5_emb, np.float32)
    contact = np.asarray(contact_matrix, np.float32)
    wc = np.asarray(Wc, np.float32).reshape(H)
    inv = 1.0 / np.sqrt(np.float32(D))

    embT = np.ascontiguousarray(emb.T)  # [IN, S]
    woTp = np.ascontiguousarray(
        np.asarray(Wo, np.float32).T.reshape(8, 64, NODE).transpose(1, 0, 2)
    ).astype(bf)
    vecs = np.ascontiguousarray(np.stack([gamma, beta]).astype(np.float32))
    bo_f = np.asarray(bo, np.float32)

    # per-sh permuted emb (contiguous chunk layout) and contactT
    embc_sh = []
    contactT_sh = []
    for sh in range(N_SH):
        own = slice(sh * SH_S, (sh + 1) * SH_S)
        oth = slice((1 - sh) * SH_S, (2 - sh) * SH_S)
        ep = np.concatenate([embT[:, own], embT[:, oth]], axis=1)  # [IN, S]
        # [c=8, p=128, sc=8, s'=512] -> [p, sc, c, s'] -> [128, 8, 4096]
        ec = ep.reshape(8, 128, 8, 512).transpose(1, 2, 0, 3).reshape(128, 8, 8 * 512)
        embc_sh.append(np.ascontiguousarray(ec).astype(bf))
        A = contact[own, :]  # [SH_S rows s, S cols t]
        contactT_sh.append(np.ascontiguousarray(
            np.concatenate([A[:, own].T, A[:, oth].T], axis=0)).astype(bf))

    in_maps = []
    for c in range(N_CORES):
        g, sh = c % N_G, c // N_G
        h0, h1 = 2 * g, 2 * g + 1
        s0, s1 = slice(h0 * D, (h0 + 1) * D), slice(h1 * D, (h1 + 1) * D)
        wqqT = np.ascontiguousarray(np.concatenate([Wq[s0], Wq[s1]]).T)
        wkkT = np.ascontiguousarray(np.concatenate([Wk[s0], Wk[s1]]).T)
        wvvT = np.ascontiguousarray(np.concatenate([Wv[s0], Wv[s1]]).T)
        qscale = np.concatenate([
            np.full(D, wc[h0] * inv, np.float32),
            np.full(D, wc[h1] * inv, np.float32)])[:, None]
        qbias = np.concatenate([
            np.asarray(bq, np.float32)[s0] * (wc[h0] * inv),
            np.asarray(bq, np.float32)[s1] * (wc[h1] * inv)])[:, None]
        kbias = np.concatenate([np.asarray(bk, np.float32)[s0],
                                np.asarray(bk, np.float32)[s1]])[:, None]
        vbias = np.concatenate([np.asarray(bv, np.float32)[s0],
                                np.asarray(bv, np.float32)[s1]])[:, None]
        r0 = slice(h0 * NODE + sh * 256, h0 * NODE + (sh + 1) * 256)
        r1 = slice(h1 * NODE + sh * 256, h1 * NODE + (sh + 1) * 256)
        emb_res = np.ascontiguousarray(
            np.concatenate([emb[r0, :NODE], emb[r1, :NODE]]) + bo_f[None, :])
        in_maps.append({
            "embc": embc_sh[sh],
            "contactT": contactT_sh[sh],
            "wqqT": wqqT.astype(bf),
            "wkkT": wkkT.astype(bf),
            "wvvT": wvvT.astype(bf),
            "qscale": np.ascontiguousarray(qscale),
            "qbias": np.ascontiguousarray(qbias),
            "kbias": np.ascontiguousarray(kbias),
            "vbias": np.ascontiguousarray(vbias),
            "woTp": woTp,
            "vecs": vecs,
            "emb_res": emb_res,
        })
    return in_maps


def assemble(results):
    out = np.empty((S, NODE), np.float32)
    for c in range(N_CORES):
        g, sh = c % N_G, c // N_G
        h0, h1 = 2 * g, 2 * g + 1
        blk = results[c]["out"]
        out[h0 * NODE + sh * 256: h0 * NODE + (sh + 1) * 256] = blk[:256]
        out[h1 * NODE + sh * 256: h1 * NODE + (sh + 1) * 256] = blk[256:]
    return out


def kernel(**inputs):
    from concourse.bass_utils import run_bass_kernel_spmd

    if "nc" not in _cache:
        _cache["nc"] = _build()
    nc = _cache["nc"]
    in_maps = make_in_maps(**inputs)
    res = run_bass_kernel_spmd(nc, in_maps, list(range(N_CORES)))
    return assemble(res.results)
